# revision 14
# baseline (speedup 1.0000x reference)
"""GCN message-passing kernel for Trainium2 (8 NeuronCores, Bass/Tile).

out = coef * relu(C_U * D^-1/2 A~^T D^-1/2 (x W^T + b)),  A~ = A + I

Strategy (dst-sharded, fully static SPMD program, 512B pair-gather):
- Core c owns a 12,500-node dst range. Host deals dsts into 96-wide
  "windows" (LPT bin-packing on in-degree), 6 windows per group.
- The SWDGE gather tax on this ucode build is ~2.1ns per DESCRIPTOR
  (4 queues, flat in call size / index locality / elem size up to
  512B), so the kernel gathers 512B descriptors that each carry TWO
  node rows: the host pairs nodes (same-window greedy matching on the
  real edge list, ~22% of (src,window) incidences covered by a
  partner) and uploads a per-core pair table xpair[k] =
  [xh[a_k] ++ xh[b_k]] (node-level permutation+concat only - no
  per-edge host gather). A descriptor for window w covers the a-half
  and/or b-half via TWO one-hot matmuls per 128-slot column.
- W commutes with aggregation: aggregate xs = dis_src*x rows first,
  apply W once per output node afterwards (dis_src folded into xh on
  host; dis_dst*coef folded into the output activation scale).
- Pair-index reach: int16 over 25,000-pair segments (2 segments);
  per-(window, segment) static quotas (max over cores, rounded to 64
  so group totals stay 128-aligned) make one program serve all cores.
- Self loops: host pre-permutes the core's own pre-scaled rows into
  window order (128-slot windows, 96 live) -> sequential DMA, no
  descriptors.
- Device per group: dma_gather 1024-desc calls round-robined on 4
  SWDGE queues; DVE builds per-(stream,half) one-hot stacks
  (iota==doff); PE accumulates ps1[D,96] per window with 2 matmuls
  per pair column (a-half, b-half stationary slices); stage-2 matmul
  applies W^T; ACT fuses relu + coef*C_U*dis_dst; DMA out.
- Host unpermutes the window-ordered output rows.
"""

import sys
import types

import numpy as np


def _install_ntff_hook_bridge():
    """antenv.axon_hooks is missing from this image; bridge it so
    run_bass_kernel_spmd(trace=True) can profile. Harmless if unused."""
    if "antenv.axon_hooks" in sys.modules:
        return
    hooks = types.ModuleType("antenv.axon_hooks")
    hooks._HOOK = None

    def _get():
        if hooks._HOOK is None:
            try:
                from trn_agent_boot.trn_boot import _ntff_profile_via_ctypes

                hooks._HOOK = _ntff_profile_via_ctypes("/opt/axon/libaxon_pjrt.so")
            except Exception:
                hooks._HOOK = None
        return hooks._HOOK

    hooks.get_axon_ntff_profile_hook = _get
    hooks.set_axon_ntff_profile_hook = lambda h: setattr(hooks, "_HOOK", h)
    sys.modules["antenv.axon_hooks"] = hooks


_install_ntff_hook_bridge()

C_SIGMA = 2.0
C_U = 1.0
PSEG = 25000  # pairs per gather segment (int16 reach 32767)
W_WIN = 96  # dst window width (one-hot width)
SELF_Q = 128  # self-stream slots per window (96 live + pad, 128-aligned)
N_CORES = 8
GROUP = 6  # windows per group (SBUF-bounded: msgs tiles are 512B/slot-col)
GATHER_CAP = 1024  # descs per dma_gather call (SWDGE ring carveout)


def _ceil(a, b):
    return (a + b - 1) // b


def _wrap16(idx, ncols):
    """[n] int16 -> [128, ncols] wrapped in 16 partitions, replicated x8."""
    n = idx.shape[0]
    out = np.zeros((16, ncols), dtype=np.int16)
    out[np.arange(n) % 16, np.arange(n) // 16] = idx
    return np.tile(out, (8, 1))


class _Prep:
    """Host-side sharding/preprocessing result."""


def prepare(x, edge_index, W, b, n_cores=N_CORES, w_win=W_WIN, group=GROUP):
    f16 = np.float16
    N, D = x.shape
    assert N % n_cores == 0
    npc = N // n_cores
    nwin = _ceil(npc, w_win)
    nwin = _ceil(nwin, group) * group  # groups tile exactly

    src = np.asarray(edge_index[0], dtype=np.int64)
    dst = np.asarray(edge_index[1], dtype=np.int64)
    deg = np.bincount(src, minlength=N).astype(np.float32) + 1.0
    dis = deg ** -0.5

    p = _Prep()
    p.N, p.D, p.npc, p.nwin = N, D, npc, nwin
    p.n_cores, p.w_win, p.group = n_cores, w_win, group
    p.coef = np.sqrt(C_SIGMA / D).astype(np.float32)
    p.xh = (dis[:, None] * np.asarray(x, dtype=np.float32)).astype(f16)

    core_of = dst // npc
    dstloc = dst - core_of * npc

    # --- per-core window assignment: LPT on total in-degree
    indeg = np.bincount(dst, minlength=N).astype(np.int64)
    p.win_members = []
    p.win_of = np.empty((n_cores, npc), dtype=np.int32)
    p.pos_of = np.empty((n_cores, npc), dtype=np.int32)
    for c in range(n_cores):
        tot = indeg[c * npc : (c + 1) * npc]
        order = np.argsort(-tot, kind="stable").astype(np.int32)
        loads = np.zeros(nwin, dtype=np.int64)
        counts = np.zeros(nwin, dtype=np.int64)
        memb = -np.ones(nwin * w_win, dtype=np.int64)
        full_pen = np.zeros(nwin, dtype=np.int64)
        for d in order:
            w = int(np.argmin(loads + full_pen))
            r = counts[w]
            counts[w] = r + 1
            if counts[w] >= w_win:
                full_pen[w] = 1 << 40
            loads[w] += tot[d]
            p.win_of[c, d] = w
            p.pos_of[c, d] = r
            memb[w * w_win + r] = d
        p.win_members.append(memb)

    e_w = p.win_of[core_of, dstloc]
    e_off = p.pos_of[core_of, dstloc]

    # --- per-core: matching, pair table, desc lists ----------------------
    p.nstream = 3  # pseg0, pseg1, self
    p.pairs = []  # per core: [N/2, 2] node ids
    p.descs = []  # per core: list over (w, seg) -> (pid_local, offA, offB)
    rng = np.random.default_rng(7)
    for c in range(n_cores):
        m = core_of == c
        cw, coff, csrc = e_w[m], e_off[m], src[m]
        # (w, s) incidences, primary offset = first edge, extras separate
        o = np.lexsort((coff, csrc, cw))
        ws, ss, os_ = cw[o], csrc[o], coff[o]
        key = ws.astype(np.int64) * N + ss
        newg = np.empty(len(key), dtype=bool)
        newg[0] = True
        newg[1:] = key[1:] != key[:-1]
        uw = ws[newg].astype(np.int64)
        usrc = ss[newg]
        uoff = os_[newg]
        ukey = key[newg]
        # same-window greedy matching over unique (w, s)
        partner = np.full(N, -1, dtype=np.int64)
        wstart = np.searchsorted(uw, np.arange(nwin))
        wend = np.searchsorted(uw, np.arange(nwin) + 1)
        for w in range(nwin):
            cand = usrc[wstart[w] : wend[w]]
            un = cand[partner[cand] < 0]
            k = len(un) // 2
            if k:
                a, bb = un[: 2 * k : 2], un[1 : 2 * k : 2]
                partner[a] = bb
                partner[bb] = a
        unm = np.where(partner < 0)[0]
        assert len(unm) % 2 == 0
        a, bb = unm[0::2], unm[1::2]
        partner[a] = bb
        partner[bb] = a
        A = np.where(np.arange(N) < partner)[0]
        pairs = np.stack([A, partner[A]], axis=1)  # [N/2, 2]
        npairs = pairs.shape[0]
        pair_id = np.empty(N, dtype=np.int64)
        half_of = np.empty(N, dtype=np.int64)
        pair_id[pairs[:, 0]] = np.arange(npairs)
        pair_id[pairs[:, 1]] = np.arange(npairs)
        half_of[pairs[:, 0]] = 0
        half_of[pairs[:, 1]] = 1

        # ownership: incidence (w,s) emits the desc if partner absent in w
        # or s < partner (partner-present case handled once)
        pkey = uw * N + partner[usrc]
        ppresent = (
            np.searchsorted(ukey, pkey) < len(ukey)
        ) & (
            ukey[np.minimum(np.searchsorted(ukey, pkey), len(ukey) - 1)] == pkey
        )
        owner = (~ppresent) | (usrc < partner[usrc])
        # partner's primary offset for shared descs
        pidx = np.searchsorted(ukey, pkey)
        poff = np.where(ppresent, uoff[np.minimum(pidx, len(ukey) - 1)], -1)

        # per-incidence desc fields (owners only)
        ow = uw[owner]
        opid = pair_id[usrc[owner]]
        ohalf = half_of[usrc[owner]]
        ooff = uoff[owner]
        opoff = poff[owner]  # partner offset or -1
        offA = np.where(ohalf == 0, ooff, opoff)
        offB = np.where(ohalf == 0, opoff, ooff)

        # extras: multi-edges beyond the primary per (w,s): own desc
        ext = ~newg
        ew_, es_, eo_ = ws[ext], ss[ext], os_[ext]
        epid = pair_id[es_]
        ehalf = half_of[es_]
        eA = np.where(ehalf == 0, eo_, -1)
        eB = np.where(ehalf == 0, -1, eo_)
        ow = np.concatenate([ow, ew_.astype(np.int64)])
        opid = np.concatenate([opid, epid])
        offA = np.concatenate([offA, eA])
        offB = np.concatenate([offB, eB])

        # pair -> segment (balanced per window): greedy by first-use window
        nseg_p = _ceil(npairs, PSEG)
        assert nseg_p == 2 and npairs == 2 * PSEG
        use_w = {}
        o2 = np.argsort(opid, kind="stable")
        spid, sw_ = opid[o2], ow[o2]
        bnd = np.searchsorted(spid, np.arange(npairs + 1))
        loads = np.zeros((2, nwin), dtype=np.int64)
        cap = [PSEG, PSEG]
        fill = [0, 0]
        pair_seg = np.full(npairs, -1, dtype=np.int64)
        nuse = bnd[1:] - bnd[:-1]
        for pid in np.argsort(-nuse, kind="stable"):
            wl = sw_[bnd[pid] : bnd[pid + 1]]
            if len(wl) == 0:
                continue
            s0 = loads[0][wl].sum()
            s1 = loads[1][wl].sum()
            s = 0 if (s0 <= s1 and fill[0] < cap[0]) else 1
            if fill[s] >= cap[s]:
                s = 1 - s
            pair_seg[pid] = s
            fill[s] += 1
            loads[s][wl] += 1
        # inactive pairs fill the remaining capacity
        for pid in np.where(pair_seg < 0)[0]:
            s = 0 if fill[0] < cap[0] else 1
            pair_seg[pid] = s
            fill[s] += 1
        # local index within segment
        loc = np.empty(npairs, dtype=np.int64)
        for s in (0, 1):
            sel = np.where(pair_seg == s)[0]
            loc[sel] = np.arange(len(sel))
        # reorder the pair table segment-major so xpair rows follow locals
        new_order = np.empty(npairs, dtype=np.int64)
        new_order[np.where(pair_seg == 0)[0]] = np.arange(fill[0])
        new_order[np.where(pair_seg == 1)[0]] = PSEG + np.arange(fill[1])
        table = np.empty((npairs, 2), dtype=np.int64)
        table[new_order] = pairs
        p.pairs.append(table)

        dsc = {}
        eseg = pair_seg[opid]
        eloc = loc[opid]
        for s in (0, 1):
            for w in range(nwin):
                msel = (eseg == s) & (ow == w)
                dsc[(w, s)] = (eloc[msel], offA[msel], offB[msel])
        p.descs.append(dsc)

    # --- quotas per (window, pseg): max over cores, rounded to 64
    q0 = max(
        len(p.descs[c][(w, 0)][0]) for c in range(n_cores) for w in range(nwin)
    )
    q1 = max(
        len(p.descs[c][(w, 1)][0]) for c in range(n_cores) for w in range(nwin)
    )
    p.quotas = [
        max(64, _ceil(q0, 64) * 64),
        max(64, _ceil(q1, 64) * 64),
        SELF_Q,
    ]

    # --- static pass schedule (window-major; col ranges straddle windows)
    p.passes = []
    p.win_passes = [[] for _ in range(nwin)]
    for w in range(nwin):
        for q in range(p.nstream):
            Q = p.quotas[q]
            c0 = (w * Q) // 128
            c1 = ((w + 1) * Q - 1) // 128
            for col in range(c0, c1 + 1):
                p.win_passes[w].append(len(p.passes))
                p.passes.append((w, q, col))
    p.npass = len(p.passes)

    p.ngroups = nwin // group
    p.group_sizes = [group] * p.ngroups
    p.gpasses = []
    for g in range(p.ngroups):
        row = []
        for q in range(p.nstream):
            Q = p.quotas[q]
            n = 0
            for w in range(g * group, (g + 1) * group):
                n += ((w + 1) * Q - 1) // 128 - (w * Q) // 128 + 1
            row.append(n)
        p.gpasses.append(row)
    p.colsmax = max(max(row) for row in p.gpasses)

    # --- per-core slot fill + packed tables
    tot_idx_cols = sum(
        sum(group * Q // 16 for Q in p.quotas[:2]) for _ in range(p.ngroups)
    )
    tot_pass = sum(sum(row) for row in p.gpasses)
    p.tot_idx_cols, p.tot_pass = tot_idx_cols, tot_pass
    p.idx_all, p.doffA_all, p.doffB_all, p.sd = [], [], [], []
    p.xpair_all, p.xself_perm = [], []

    for c in range(n_cores):
        table = p.pairs[c]
        xp = np.empty((table.shape[0], 2 * D), dtype=f16)
        xp[:, :D] = p.xh[table[:, 0]]
        xp[:, D:] = p.xh[table[:, 1]]
        p.xpair_all.append(xp)

        # slot arrays per gather stream
        sl_idx, sl_a, sl_b = [], [], []
        for s in (0, 1):
            Q = p.quotas[s]
            S = nwin * Q
            idx16 = ((np.arange(S, dtype=np.int64) * 7919) % PSEG).astype(
                np.int16
            )
            av = -np.ones(S, dtype=np.float32)
            bv = -np.ones(S, dtype=np.float32)
            for w in range(nwin):
                el, ea, eb = p.descs[c][(w, s)]
                o = np.argsort(el, kind="stable")
                el, ea, eb = el[o], ea[o], eb[o]
                base = w * Q
                idx16[base : base + len(el)] = el.astype(np.int16)
                av[base : base + len(el)] = ea
                bv[base : base + len(el)] = eb
            sl_idx.append(idx16)
            sl_a.append(av)
            sl_b.append(bv)
        # self stream: host pre-permutes own pre-scaled rows window-major
        memb = p.win_members[c]
        Q = SELF_Q
        S = nwin * Q
        av = -np.ones(S, dtype=np.float32)
        real = memb >= 0
        wslots = np.arange(nwin * w_win)
        slots = (wslots // w_win) * Q + (wslots % w_win)
        slots = slots[real]
        nodes = memb[real]
        av[slots] = wslots[real] % w_win
        sl_idx.append(np.zeros(S, dtype=np.int16))
        sl_a.append(av)
        sl_b.append(-np.ones(S, dtype=np.float32))
        xsp = np.zeros((128, S // 128, D), dtype=f16)
        xsp[slots % 128, slots // 128] = p.xh[c * npc + nodes]
        p.xself_perm.append(xsp)

        # pack group-major
        idx_cols = np.zeros((128, tot_idx_cols), dtype=np.int16)
        dA = np.zeros((128, tot_pass), dtype=f16)
        dB = np.zeros((128, tot_pass), dtype=f16)
        ic = 0
        pc_i = 0
        for g in range(p.ngroups):
            w0 = g * group
            for q in (0, 1):
                Q = p.quotas[q]
                seg_slots = sl_idx[q][w0 * Q : (w0 + group) * Q]
                ncol = group * Q // 16
                idx_cols[:, ic : ic + ncol] = _wrap16(seg_slots, ncol)
                ic += ncol
            for q in range(p.nstream):
                Q = p.quotas[q]
                for w in range(w0, w0 + group):
                    c0 = (w * Q) // 128
                    c1 = ((w + 1) * Q - 1) // 128
                    for col in range(c0, c1 + 1):
                        s0 = col * 128
                        sl = np.arange(s0, s0 + 128)
                        inw = (sl >= w * Q) & (sl < (w + 1) * Q)
                        va = np.full(128, -1.0, dtype=np.float32)
                        vb = np.full(128, -1.0, dtype=np.float32)
                        va[inw] = sl_a[q][sl[inw]]
                        vb[inw] = sl_b[q][sl[inw]]
                        dA[:, pc_i] = va.astype(f16)
                        dB[:, pc_i] = vb.astype(f16)
                        pc_i += 1
        assert pc_i == tot_pass and ic == tot_idx_cols
        p.idx_all.append(idx_cols)
        p.doffA_all.append(dA)
        p.doffB_all.append(dB)

        sdv = np.zeros((w_win, nwin), dtype=np.float32)
        nodes_per_win = memb.reshape(nwin, w_win)
        for w in range(nwin):
            mm = nodes_per_win[w] >= 0
            sdv[mm, w] = (
                p.coef * C_U * dis[c * npc + nodes_per_win[w][mm]]
            ).astype(np.float32)
        p.sd.append(sdv)

    # pass counts per (group, stream) for vh sizing
    p.iota = np.ascontiguousarray(
        np.broadcast_to(
            np.arange(w_win, dtype=np.float32)[None, :, None],
            (128, w_win, p.colsmax),
        ).astype(f16)
    )
    p.WT16 = np.ascontiguousarray(np.asarray(W, dtype=np.float32).T).astype(f16)
    p.b = np.asarray(b, dtype=np.float32)
    p.bias_nonzero = bool(np.any(p.b != 0))
    if p.bias_nonzero:
        sb = np.zeros((n_cores, nwin * w_win), dtype=np.float32)
        np.add.at(sb, (core_of, e_w * w_win + e_off), dis[src])
        for c in range(n_cores):
            memb = p.win_members[c]
            real = memb >= 0
            slots = np.arange(nwin * w_win)[real]
            sb[c, slots] += dis[c * npc + memb[real]]
        p.sb = sb.reshape(n_cores, 1, nwin * w_win).astype(f16)
        p.b16 = p.b.reshape(1, -1).astype(f16)
    return p


def build_program(p):
    import concourse.bacc as bacc
    import concourse.mybir as mybir
    import concourse.tile as tile

    f32, f16i, i16 = mybir.dt.float32, mybir.dt.float16, mybir.dt.int16
    D, nwin, group = p.D, p.nwin, p.group
    nstream = p.nstream

    nc = bacc.Bacc(
        "TRN2", target_bir_lowering=False, debug=False, num_swdge_queues=4
    )
    xpair_d = nc.dram_tensor("xpair", [2 * PSEG, 2 * D], f16i, kind="ExternalInput")
    xself_d = nc.dram_tensor(
        "xself", [128, p.nwin * SELF_Q // 128, D], f16i, kind="ExternalInput"
    )
    wt_d = nc.dram_tensor("wt", [D, D], f16i, kind="ExternalInput")
    iota_d = nc.dram_tensor(
        "iota", [128, p.w_win, p.colsmax], f16i, kind="ExternalInput"
    )
    idx_d = nc.dram_tensor("idx", [128, p.tot_idx_cols], i16, kind="ExternalInput")
    dA_d = nc.dram_tensor("doffA", [128, p.tot_pass], f16i, kind="ExternalInput")
    dB_d = nc.dram_tensor("doffB", [128, p.tot_pass], f16i, kind="ExternalInput")
    sd_d = nc.dram_tensor("sd", [p.w_win, nwin], f32, kind="ExternalInput")
    if p.bias_nonzero:
        sb_d = nc.dram_tensor("sb", [1, nwin * p.w_win], f16i, kind="ExternalInput")
        b_d = nc.dram_tensor("b", [1, D], f16i, kind="ExternalInput")
    out_d = nc.dram_tensor("out", [p.w_win, nwin, D], f32, kind="ExternalOutput")

    segs = [xpair_d[0:PSEG, :], xpair_d[PSEG : 2 * PSEG, :], None]

    with tile.TileContext(nc) as tc:
        with (
            tc.tile_pool(name="const", bufs=1) as constp,
            tc.tile_pool(name="meta", bufs=4) as metap,
            tc.tile_pool(name="msgs", bufs=2) as msgsp,
            tc.tile_pool(name="vh", bufs=2) as vhp,
            tc.tile_pool(name="aggx", bufs=3) as aggxp,
            tc.tile_pool(name="outsb", bufs=2) as outp,
            tc.tile_pool(name="ps1", bufs=4, space="PSUM") as ps1p,
            tc.tile_pool(name="ps2", bufs=2, space="PSUM") as ps2p,
        ):
            wt16 = constp.tile([D, D], f16i, tag="wt16")
            nc.scalar.dma_start(wt16[:], wt_d[:])
            iota_sb = constp.tile([128, p.w_win, p.colsmax], f16i, tag="iota")
            nc.scalar.dma_start(iota_sb[:], iota_d[:])
            sd_sb = constp.tile([p.w_win, nwin], f32, tag="sd")
            nc.scalar.dma_start(sd_sb[:], sd_d[:])
            if p.bias_nonzero:
                b16 = constp.tile([1, D], f16i, tag="b16")
                nc.scalar.dma_start(b16[:], b_d[:])
                sbrow16 = constp.tile([1, nwin * p.w_win], f16i, tag="sbw16")
                nc.scalar.dma_start(sbrow16[:], sb_d[:])

            ic_base = 0
            pass_base = 0
            gq = [0]
            for g in range(p.ngroups):
                w0 = g * group
                gidx_cols = sum(group * Q // 16 for Q in p.quotas[:2])
                gpass = sum(p.gpasses[g])
                idx_sb = metap.tile([128, gidx_cols], i16, tag="idx")
                nc.sync.dma_start(idx_sb[:], idx_d[:, ic_base : ic_base + gidx_cols])
                dA_sb = metap.tile([128, gpass], f16i, tag="dA")
                nc.sync.dma_start(dA_sb[:], dA_d[:, pass_base : pass_base + gpass])
                dB_sb = metap.tile([128, gpass], f16i, tag="dB")
                nc.sync.dma_start(dB_sb[:], dB_d[:, pass_base : pass_base + gpass])

                ms = []
                vhA = []
                vhB = []
                icol = 0
                ppos = 0
                for q in range(nstream):
                    Q = p.quotas[q]
                    npas = p.gpasses[g][q]
                    if q < 2:
                        ncols = group * Q // 128
                        mt = msgsp.tile([128, ncols, 2 * D], f16i, tag=f"m{q}")
                        off = 0
                        total = group * Q
                        while off < total:
                            n = min(GATHER_CAP, total - off)
                            nc.gpsimd.dma_gather(
                                mt[:, off // 128 : (off + n) // 128, :],
                                segs[q],
                                idx_sb[
                                    :, icol + off // 16 : icol + (off + n) // 16
                                ],
                                n,
                                n,
                                2 * D,
                                queue_num=gq[0] % 4,
                                single_packet=False,
                            )
                            gq[0] += 1
                            off += n
                        icol += group * Q // 16
                    else:
                        ncols = group * Q // 128
                        mt = msgsp.tile([128, ncols, D], f16i, tag="mself")
                        c0 = w0 * Q // 128
                        nc.sync.dma_start(mt[:], xself_d[:, c0 : c0 + ncols, :])
                    ms.append(mt)

                    def _bcast(ap2d, n=npas):
                        return ap2d.rearrange("p (o c) -> p o c", o=1).broadcast_to(
                            [128, p.w_win, n]
                        )

                    va = vhp.tile([128, p.w_win, npas], f16i, tag=f"va{q}")
                    nc.vector.tensor_tensor(
                        va[:],
                        iota_sb[:, :, :npas],
                        _bcast(dA_sb[:, ppos : ppos + npas]),
                        mybir.AluOpType.is_equal,
                    )
                    vhA.append(va)
                    if q < 2:
                        vb = vhp.tile([128, p.w_win, npas], f16i, tag=f"vb{q}")
                        nc.vector.tensor_tensor(
                            vb[:],
                            iota_sb[:, :, :npas],
                            _bcast(dB_sb[:, ppos : ppos + npas]),
                            mybir.AluOpType.is_equal,
                        )
                        vhB.append(vb)
                    else:
                        vhB.append(None)
                    ppos += npas

                out_sb = outp.tile([p.w_win, group, D], f32, tag="out")
                pass_ctr = [0] * nstream
                for wl in range(group):
                    w = w0 + wl
                    ps1 = ps1p.tile([D, p.w_win], f32, tag="ps1")
                    plist = p.win_passes[w]
                    # expand to (q, col_local, half) matmul list
                    mms = []
                    for pi in plist:
                        _, q, col = p.passes[pi]
                        Q = p.quotas[q]
                        col_l = col - (w0 * Q) // 128
                        pl = pass_ctr[q]
                        pass_ctr[q] += 1
                        mms.append((q, col_l, pl, 0))
                        if q < 2:
                            mms.append((q, col_l, pl, 1))
                    for k, (q, col_l, pl, hf) in enumerate(mms):
                        if q < 2:
                            stat = ms[q][:, col_l, hf * D : (hf + 1) * D]
                            vt = vhA[q] if hf == 0 else vhB[q]
                        else:
                            stat = ms[q][:, col_l, :]
                            vt = vhA[q]
                        nc.tensor.matmul(
                            ps1[:, :],
                            stat,
                            vt[:, :, pl],
                            start=(k == 0),
                            stop=(k == len(mms) - 1),
                        )
                    # pass_ctr counted per window loop; shared straddle cols
                    # advance it once per (q, col) occurrence in plist
                    ag = aggxp.tile([D, p.w_win], f16i, tag="ag")
                    nc.scalar.copy(ag[:], ps1[:])
                    ps2 = ps2p.tile([p.w_win, D], f32, tag="ps2")
                    nc.tensor.matmul(
                        ps2[:, :],
                        ag[:, :],
                        wt16[:, :],
                        start=True,
                        stop=not p.bias_nonzero,
                    )
                    if p.bias_nonzero:
                        nc.tensor.matmul(
                            ps2[:, :],
                            sbrow16[:, w * p.w_win : (w + 1) * p.w_win],
                            b16[:, :],
                            start=False,
                            stop=True,
                        )
                    nc.scalar.activation(
                        out_sb[:, wl, :],
                        ps2[:, :],
                        mybir.ActivationFunctionType.Relu,
                        scale=sd_sb[:, w : w + 1],
                    )
                nc.sync.dma_start(out_d[:, w0 : w0 + group, :], out_sb[:])
                ic_base += gidx_cols
                pass_base += gpass
    nc.compile()
    return nc


def _unshard(p, outs):
    N, D = p.N, p.D
    res = np.empty((N, D), dtype=np.float32)
    for c in range(p.n_cores):
        o = np.asarray(outs[c]).transpose(1, 0, 2).reshape(p.nwin * p.w_win, D)
        memb = p.win_members[c]
        real = memb >= 0
        res[c * p.npc + memb[real]] = o[real]
    return res


def _in_maps(p):
    in_maps = []
    for c in range(p.n_cores):
        m = {
            "xpair": p.xpair_all[c],
            "xself": p.xself_perm[c],
            "wt": p.WT16,
            "iota": p.iota,
            "idx": p.idx_all[c],
            "doffA": p.doffA_all[c],
            "doffB": p.doffB_all[c],
            "sd": p.sd[c],
        }
        if p.bias_nonzero:
            m["sb"] = p.sb[c]
            m["b"] = p.b16
        in_maps.append(m)
    return in_maps


def kernel(x, edge_index, W, b):
    from concourse.bass_utils import run_bass_kernel_spmd

    x = np.asarray(x, dtype=np.float32)
    W = np.asarray(W, dtype=np.float32)
    b = np.asarray(b, dtype=np.float32)
    p = prepare(x, edge_index, W, b)
    nc = build_program(p)
    res = run_bass_kernel_spmd(nc, _in_maps(p), core_ids=list(range(p.n_cores)))
    outs = [r["out"] for r in res.results]
    return _unshard(p, outs)


# revision 24
# speedup vs baseline: 1.8256x; 1.8256x over previous
"""GCN message-passing kernel for Trainium2 (8 NeuronCores, Bass/Tile).

out = coef * relu(C_U * D^-1/2 A~^T D^-1/2 (x W^T + b)),  A~ = A + I

Strategy (dst-sharded, fully static SPMD program, 512B pair-gather):
- Core c owns a 12,500-node dst range. Host deals dsts into 96-wide
  "windows" (LPT bin-packing on in-degree), 6 windows per group.
- The SWDGE gather tax on this ucode build is ~2.1ns per DESCRIPTOR
  (4 queues, flat in call size / index locality / elem size up to
  512B), so the kernel gathers 512B descriptors that each carry TWO
  node rows: the host pairs nodes (same-window greedy matching on the
  real edge list, ~22% of (src,window) incidences covered by a
  partner) and uploads a per-core pair table xpair[k] =
  [xh[a_k] ++ xh[b_k]] (node-level permutation+concat only - no
  per-edge host gather). A descriptor for window w covers the a-half
  and/or b-half via TWO one-hot matmuls per 128-slot column.
- W commutes with aggregation: aggregate xs = dis_src*x rows first,
  apply W once per output node afterwards (dis_src folded into xh on
  host; dis_dst*coef folded into the output activation scale).
- Pair-index reach: int16 over 25,000-pair segments (2 segments);
  per-(window, segment) static quotas (max over cores, rounded to 64
  so group totals stay 128-aligned) make one program serve all cores.
- Self loops: host pre-permutes the core's own pre-scaled rows into
  window order (128-slot windows, 96 live) -> sequential DMA, no
  descriptors.
- Device per group: dma_gather 1024-desc calls round-robined on 4
  SWDGE queues; DVE builds per-(stream,half) one-hot stacks
  (iota==doff); PE accumulates ps1[D,96] per window with 2 matmuls
  per pair column (a-half, b-half stationary slices); stage-2 matmul
  applies W^T; ACT fuses relu + coef*C_U*dis_dst; DMA out.
- Host unpermutes the window-ordered output rows.
"""

import sys
import types

import numpy as np


def _install_ntff_hook_bridge():
    """antenv.axon_hooks is missing from this image; bridge it so
    run_bass_kernel_spmd(trace=True) can profile. Harmless if unused."""
    if "antenv.axon_hooks" in sys.modules:
        return
    hooks = types.ModuleType("antenv.axon_hooks")
    hooks._HOOK = None

    def _get():
        if hooks._HOOK is None:
            try:
                from trn_agent_boot.trn_boot import _ntff_profile_via_ctypes

                hooks._HOOK = _ntff_profile_via_ctypes("/opt/axon/libaxon_pjrt.so")
            except Exception:
                hooks._HOOK = None
        return hooks._HOOK

    hooks.get_axon_ntff_profile_hook = _get
    hooks.set_axon_ntff_profile_hook = lambda h: setattr(hooks, "_HOOK", h)
    sys.modules["antenv.axon_hooks"] = hooks


_install_ntff_hook_bridge()

C_SIGMA = 2.0
C_U = 1.0
PSEG = 25000  # pairs per gather segment (int16 reach 32767)
W_WIN = 96  # dst window width (one-hot width)
SELF_Q = 128  # self-stream slots per window (96 live + pad, 128-aligned)
N_CORES = 8
GROUP = 6  # windows per group (SBUF-bounded: msgs tiles are 512B/slot-col)
GATHER_CAP = 1024  # descs per dma_gather call (SWDGE ring carveout)


def _ceil(a, b):
    return (a + b - 1) // b


def _wrap16(idx, ncols):
    """[n] int16 -> [128, ncols] wrapped in 16 partitions, replicated x8."""
    n = idx.shape[0]
    out = np.zeros((16, ncols), dtype=np.int16)
    out[np.arange(n) % 16, np.arange(n) // 16] = idx
    return np.tile(out, (8, 1))


class _Prep:
    """Host-side sharding/preprocessing result."""


def prepare(x, edge_index, W, b, n_cores=N_CORES, w_win=W_WIN, group=GROUP):
    f16 = np.float16
    N, D = x.shape
    assert N % n_cores == 0
    npc = N // n_cores
    nwin = _ceil(npc, w_win)
    nwin = _ceil(nwin, group) * group  # groups tile exactly

    src = np.asarray(edge_index[0], dtype=np.int64)
    dst = np.asarray(edge_index[1], dtype=np.int64)
    deg = np.bincount(src, minlength=N).astype(np.float32) + 1.0
    dis = deg ** -0.5

    p = _Prep()
    p.N, p.D, p.npc, p.nwin = N, D, npc, nwin
    p.n_cores, p.w_win, p.group = n_cores, w_win, group
    p.coef = np.sqrt(C_SIGMA / D).astype(np.float32)
    p.xh = (dis[:, None] * np.asarray(x, dtype=np.float32)).astype(f16)

    core_of = dst // npc
    dstloc = dst - core_of * npc

    # --- per-core window assignment: LPT on total in-degree
    indeg = np.bincount(dst, minlength=N).astype(np.int64)
    p.win_members = []
    p.win_of = np.empty((n_cores, npc), dtype=np.int32)
    p.pos_of = np.empty((n_cores, npc), dtype=np.int32)
    for c in range(n_cores):
        tot = indeg[c * npc : (c + 1) * npc]
        order = np.argsort(-tot, kind="stable").astype(np.int32)
        loads = np.zeros(nwin, dtype=np.int64)
        counts = np.zeros(nwin, dtype=np.int64)
        memb = -np.ones(nwin * w_win, dtype=np.int64)
        full_pen = np.zeros(nwin, dtype=np.int64)
        for d in order:
            w = int(np.argmin(loads + full_pen))
            r = counts[w]
            counts[w] = r + 1
            if counts[w] >= w_win:
                full_pen[w] = 1 << 40
            loads[w] += tot[d]
            p.win_of[c, d] = w
            p.pos_of[c, d] = r
            memb[w * w_win + r] = d
        p.win_members.append(memb)

    e_w = p.win_of[core_of, dstloc]
    e_off = p.pos_of[core_of, dstloc]

    # --- per-core: matching, pair table, desc lists ----------------------
    p.nstream = 3  # pseg0, pseg1, self
    p.pairs = []  # per core: [N/2, 2] node ids
    p.descs = []  # per core: list over (w, seg) -> (pid_local, offA, offB)
    rng = np.random.default_rng(7)
    for c in range(n_cores):
        m = core_of == c
        cw, coff, csrc = e_w[m], e_off[m], src[m]
        # (w, s) incidences, primary offset = first edge, extras separate
        o = np.lexsort((coff, csrc, cw))
        ws, ss, os_ = cw[o], csrc[o], coff[o]
        key = ws.astype(np.int64) * N + ss
        newg = np.empty(len(key), dtype=bool)
        newg[0] = True
        newg[1:] = key[1:] != key[:-1]
        uw = ws[newg].astype(np.int64)
        usrc = ss[newg]
        uoff = os_[newg]
        ukey = key[newg]
        # matching, overlap-2 first: pair srcs sharing TWO windows (each
        # shared window saves one descriptor), then same-window greedy.
        partner = np.full(N, -1, dtype=np.int64)
        o2s = np.argsort(usrc, kind="stable")
        su, wu = usrc[o2s], uw[o2s]
        sb_ = np.searchsorted(su, np.arange(N))
        se_ = np.searchsorted(su, np.arange(N) + 1)
        cnts = se_ - sb_
        multi = np.where(cnts >= 2)[0]
        keys = []
        ksrc = []
        for s_ in multi:
            wl = wu[sb_[s_] : se_[s_]]
            for i in range(len(wl)):
                for j in range(i + 1, len(wl)):
                    keys.append(wl[i] * nwin + wl[j])
                    ksrc.append(s_)
        keys = np.asarray(keys, dtype=np.int64)
        ksrc = np.asarray(ksrc, dtype=np.int64)
        ko = np.argsort(keys, kind="stable")
        keys, ksrc = keys[ko], ksrc[ko]
        bndk = np.flatnonzero(
            np.concatenate(([True], keys[1:] != keys[:-1], [True]))
        )
        for bi in range(len(bndk) - 1):
            grp = ksrc[bndk[bi] : bndk[bi + 1]]
            un = grp[partner[grp] < 0]
            un = np.unique(un)
            k = len(un) // 2
            if k:
                a, bb = un[: 2 * k : 2], un[1 : 2 * k : 2]
                partner[a] = bb
                partner[bb] = a
        wstart = np.searchsorted(uw, np.arange(nwin))
        wend = np.searchsorted(uw, np.arange(nwin) + 1)
        for w in range(nwin):
            cand = usrc[wstart[w] : wend[w]]
            un = cand[partner[cand] < 0]
            k = len(un) // 2
            if k:
                a, bb = un[: 2 * k : 2], un[1 : 2 * k : 2]
                partner[a] = bb
                partner[bb] = a
        unm = np.where(partner < 0)[0]
        assert len(unm) % 2 == 0
        a, bb = unm[0::2], unm[1::2]
        partner[a] = bb
        partner[bb] = a
        A = np.where(np.arange(N) < partner)[0]
        pairs = np.stack([A, partner[A]], axis=1)  # [N/2, 2]
        npairs = pairs.shape[0]
        pair_id = np.empty(N, dtype=np.int64)
        half_of = np.empty(N, dtype=np.int64)
        pair_id[pairs[:, 0]] = np.arange(npairs)
        pair_id[pairs[:, 1]] = np.arange(npairs)
        half_of[pairs[:, 0]] = 0
        half_of[pairs[:, 1]] = 1

        # ownership: incidence (w,s) emits the desc if partner absent in w
        # or s < partner (partner-present case handled once)
        pkey = uw * N + partner[usrc]
        ppresent = (
            np.searchsorted(ukey, pkey) < len(ukey)
        ) & (
            ukey[np.minimum(np.searchsorted(ukey, pkey), len(ukey) - 1)] == pkey
        )
        owner = (~ppresent) | (usrc < partner[usrc])
        # partner's primary offset for shared descs
        pidx = np.searchsorted(ukey, pkey)
        poff = np.where(ppresent, uoff[np.minimum(pidx, len(ukey) - 1)], -1)

        # per-incidence desc fields (owners only)
        ow = uw[owner]
        opid = pair_id[usrc[owner]]
        ohalf = half_of[usrc[owner]]
        ooff = uoff[owner]
        opoff = poff[owner]  # partner offset or -1
        offA = np.where(ohalf == 0, ooff, opoff)
        offB = np.where(ohalf == 0, opoff, ooff)

        # extras: multi-edges beyond the primary per (w,s): own desc
        ext = ~newg
        ew_, es_, eo_ = ws[ext], ss[ext], os_[ext]
        epid = pair_id[es_]
        ehalf = half_of[es_]
        eA = np.where(ehalf == 0, eo_, -1)
        eB = np.where(ehalf == 0, -1, eo_)
        ow = np.concatenate([ow, ew_.astype(np.int64)])
        opid = np.concatenate([opid, epid])
        offA = np.concatenate([offA, eA])
        offB = np.concatenate([offB, eB])

        # pair -> segment (balanced per window): greedy by first-use window
        nseg_p = _ceil(npairs, PSEG)
        assert nseg_p == 2 and npairs == 2 * PSEG
        use_w = {}
        o2 = np.argsort(opid, kind="stable")
        spid, sw_ = opid[o2], ow[o2]
        bnd = np.searchsorted(spid, np.arange(npairs + 1))
        loads = np.zeros((2, nwin), dtype=np.int64)
        cap = [PSEG, PSEG]
        fill = [0, 0]
        pair_seg = np.full(npairs, -1, dtype=np.int64)
        nuse = bnd[1:] - bnd[:-1]
        for pid in np.argsort(-nuse, kind="stable"):
            wl = sw_[bnd[pid] : bnd[pid + 1]]
            if len(wl) == 0:
                continue
            s0 = loads[0][wl].sum()
            s1 = loads[1][wl].sum()
            s = 0 if (s0 <= s1 and fill[0] < cap[0]) else 1
            if fill[s] >= cap[s]:
                s = 1 - s
            pair_seg[pid] = s
            fill[s] += 1
            loads[s][wl] += 1
        # inactive pairs fill the remaining capacity
        for pid in np.where(pair_seg < 0)[0]:
            s = 0 if fill[0] < cap[0] else 1
            pair_seg[pid] = s
            fill[s] += 1
        # local rebalance: flip pairs out of overloaded (seg, window) cells
        wo_ = np.argsort(sw_, kind="stable")
        w_pids = spid[wo_]
        w_bnd = np.searchsorted(sw_[wo_], np.arange(nwin + 1))
        for _sweep in range(6):
            thr = int(loads.max()) - 8
            if thr <= int(np.ceil(loads.mean())) + 8:
                break
            hot = np.argwhere(loads > thr)
            moved = 0
            for s, w in hot:
                t = 1 - s
                for pid in w_pids[w_bnd[w] : w_bnd[w + 1]]:
                    if loads[s][w] <= thr:
                        break
                    if pair_seg[pid] != s or fill[t] >= cap[t]:
                        continue
                    wl = sw_[bnd[pid] : bnd[pid + 1]]
                    if np.all(loads[t][wl] + 1 <= thr):
                        pair_seg[pid] = t
                        fill[s] -= 1
                        fill[t] += 1
                        loads[s][wl] -= 1
                        loads[t][wl] += 1
                        moved += 1
            if moved == 0:
                break
        # local index within segment
        loc = np.empty(npairs, dtype=np.int64)
        for s in (0, 1):
            sel = np.where(pair_seg == s)[0]
            loc[sel] = np.arange(len(sel))
        # reorder the pair table segment-major so xpair rows follow locals
        new_order = np.empty(npairs, dtype=np.int64)
        new_order[np.where(pair_seg == 0)[0]] = np.arange(fill[0])
        new_order[np.where(pair_seg == 1)[0]] = PSEG + np.arange(fill[1])
        table = np.empty((npairs, 2), dtype=np.int64)
        table[new_order] = pairs
        p.pairs.append(table)

        dsc = {}
        eseg = pair_seg[opid]
        eloc = loc[opid]
        for s in (0, 1):
            for w in range(nwin):
                msel = (eseg == s) & (ow == w)
                dsc[(w, s)] = (eloc[msel], offA[msel], offB[msel])
        p.descs.append(dsc)

    # --- per-core window relabel (sort by desc load): window-index k is the
    # k-th heaviest in EVERY core, so per-window quotas are near-tight maxima
    for c in range(n_cores):
        tot = np.array(
            [
                len(p.descs[c][(w, 0)][0]) + len(p.descs[c][(w, 1)][0])
                for w in range(nwin)
            ]
        )
        perm = np.argsort(-tot, kind="stable")  # new k -> old w
        inv = np.empty(nwin, dtype=np.int64)
        inv[perm] = np.arange(nwin)
        mask = core_of == c
        e_w[mask] = inv[e_w[mask]]
        p.win_of[c] = inv[p.win_of[c]]
        p.win_members[c] = (
            p.win_members[c].reshape(nwin, w_win)[perm].reshape(-1)
        )
        p.descs[c] = {
            (int(inv[w]), s): p.descs[c][(w, s)]
            for w in range(nwin)
            for s in (0, 1)
        }

    # --- per-window quotas: max over cores; group totals padded to 128
    Qw = np.zeros((nwin, 2), dtype=np.int64)
    for w in range(nwin):
        for s in (0, 1):
            Qw[w, s] = max(len(p.descs[c][(w, s)][0]) for c in range(n_cores))
    for s in (0, 1):
        for g in range(nwin // group):
            tot = int(Qw[g * group : (g + 1) * group, s].sum())
            Qw[(g + 1) * group - 1, s] += (-tot) % 128
    p.Qw = Qw
    p.B = np.zeros((2, nwin + 1), dtype=np.int64)
    for s in (0, 1):
        p.B[s, 1:] = np.cumsum(Qw[:, s])
    p.quotas = [int(Qw[:, 0].sum()), int(Qw[:, 1].sum()), SELF_Q]  # debug

    def _bounds(q, w):
        if q < 2:
            return int(p.B[q, w]), int(p.B[q, w + 1])
        return w * SELF_Q, (w + 1) * SELF_Q

    # --- static pass schedule (window-major; col ranges straddle windows)
    p.passes = []
    p.win_passes = [[] for _ in range(nwin)]
    for w in range(nwin):
        for q in range(p.nstream):
            b0, b1 = _bounds(q, w)
            if b1 == b0:
                continue
            for col in range(b0 // 128, (b1 - 1) // 128 + 1):
                p.win_passes[w].append(len(p.passes))
                p.passes.append((w, q, col))
    p.npass = len(p.passes)
    p.bounds = _bounds

    p.ngroups = nwin // group
    p.group_sizes = [group] * p.ngroups
    p.gpasses = []
    for g in range(p.ngroups):
        row = []
        for q in range(p.nstream):
            n = 0
            for w in range(g * group, (g + 1) * group):
                b0, b1 = _bounds(q, w)
                if b1 > b0:
                    n += (b1 - 1) // 128 - b0 // 128 + 1
            row.append(n)
        p.gpasses.append(row)
    p.colsmax = max(max(row) for row in p.gpasses)

    # --- per-core slot fill + packed tables
    tot_idx_cols = int(p.B[0, nwin] + p.B[1, nwin]) // 16
    tot_pass = sum(sum(row) for row in p.gpasses)
    p.tot_idx_cols, p.tot_pass = tot_idx_cols, tot_pass
    p.idx_all, p.doffA_all, p.doffB_all, p.sd = [], [], [], []
    p.xpair_all, p.xself_perm = [], []

    for c in range(n_cores):
        table = p.pairs[c]
        xp = np.empty((table.shape[0], 2 * D), dtype=f16)
        xp[:, :D] = p.xh[table[:, 0]]
        xp[:, D:] = p.xh[table[:, 1]]
        p.xpair_all.append(xp)

        # slot arrays per gather stream
        sl_idx, sl_a, sl_b = [], [], []
        for s in (0, 1):
            S = int(p.B[s, nwin])
            idx16 = ((np.arange(S, dtype=np.int64) * 7919) % PSEG).astype(
                np.int16
            )
            av = -np.ones(S, dtype=np.float32)
            bv = -np.ones(S, dtype=np.float32)
            for w in range(nwin):
                el, ea, eb = p.descs[c][(w, s)]
                o = np.argsort(el, kind="stable")
                el, ea, eb = el[o], ea[o], eb[o]
                base = int(p.B[s, w])
                idx16[base : base + len(el)] = el.astype(np.int16)
                av[base : base + len(el)] = ea
                bv[base : base + len(el)] = eb
            sl_idx.append(idx16)
            sl_a.append(av)
            sl_b.append(bv)
        # self stream: host pre-permutes own pre-scaled rows window-major
        memb = p.win_members[c]
        Q = SELF_Q
        S = nwin * Q
        av = -np.ones(S, dtype=np.float32)
        real = memb >= 0
        wslots = np.arange(nwin * w_win)
        slots = (wslots // w_win) * Q + (wslots % w_win)
        slots = slots[real]
        nodes = memb[real]
        av[slots] = wslots[real] % w_win
        sl_idx.append(np.zeros(S, dtype=np.int16))
        sl_a.append(av)
        sl_b.append(-np.ones(S, dtype=np.float32))
        xsp = np.zeros((128, S // 128, D), dtype=f16)
        xsp[slots % 128, slots // 128] = p.xh[c * npc + nodes]
        p.xself_perm.append(xsp)

        # pack group-major
        idx_cols = np.zeros((128, tot_idx_cols), dtype=np.int16)
        dA = np.zeros((128, tot_pass), dtype=f16)
        dB = np.zeros((128, tot_pass), dtype=f16)
        ic = 0
        pc_i = 0
        for g in range(p.ngroups):
            w0 = g * group
            for q in (0, 1):
                lo, hi = int(p.B[q, w0]), int(p.B[q, w0 + group])
                seg_slots = sl_idx[q][lo:hi]
                ncol = (hi - lo) // 16
                idx_cols[:, ic : ic + ncol] = _wrap16(seg_slots, ncol)
                ic += ncol
            for q in range(p.nstream):
                for w in range(w0, w0 + group):
                    b0, b1 = p.bounds(q, w)
                    if b1 == b0:
                        continue
                    for col in range(b0 // 128, (b1 - 1) // 128 + 1):
                        s0 = col * 128
                        sl = np.arange(s0, s0 + 128)
                        inw = (sl >= b0) & (sl < b1)
                        va = np.full(128, -1.0, dtype=np.float32)
                        vb = np.full(128, -1.0, dtype=np.float32)
                        va[inw] = sl_a[q][sl[inw]]
                        vb[inw] = sl_b[q][sl[inw]]
                        dA[:, pc_i] = va.astype(f16)
                        dB[:, pc_i] = vb.astype(f16)
                        pc_i += 1
        assert pc_i == tot_pass and ic == tot_idx_cols
        p.idx_all.append(idx_cols)
        p.doffA_all.append(dA)
        p.doffB_all.append(dB)

        sdv = np.zeros((w_win, nwin), dtype=np.float32)
        nodes_per_win = memb.reshape(nwin, w_win)
        for w in range(nwin):
            mm = nodes_per_win[w] >= 0
            sdv[mm, w] = (
                p.coef * C_U * dis[c * npc + nodes_per_win[w][mm]]
            ).astype(np.float32)
        p.sd.append(sdv)

    # pass counts per (group, stream) for vh sizing
    p.iota = np.ascontiguousarray(
        np.broadcast_to(
            np.arange(w_win, dtype=np.float32)[None, :, None],
            (128, w_win, p.colsmax),
        ).astype(f16)
    )
    p.WT16 = np.ascontiguousarray(np.asarray(W, dtype=np.float32).T).astype(f16)
    p.b = np.asarray(b, dtype=np.float32)
    p.bias_nonzero = bool(np.any(p.b != 0))
    if p.bias_nonzero:
        sb = np.zeros((n_cores, nwin * w_win), dtype=np.float32)
        np.add.at(sb, (core_of, e_w * w_win + e_off), dis[src])
        for c in range(n_cores):
            memb = p.win_members[c]
            real = memb >= 0
            slots = np.arange(nwin * w_win)[real]
            sb[c, slots] += dis[c * npc + memb[real]]
        p.sb = sb.reshape(n_cores, 1, nwin * w_win).astype(f16)
        p.b16 = p.b.reshape(1, -1).astype(f16)
    return p


def build_program(p):
    import concourse.bacc as bacc
    import concourse.mybir as mybir
    import concourse.tile as tile

    f32, f16i, i16 = mybir.dt.float32, mybir.dt.float16, mybir.dt.int16
    D, nwin, group = p.D, p.nwin, p.group
    nstream = p.nstream

    nc = bacc.Bacc(
        "TRN2", target_bir_lowering=False, debug=False, num_swdge_queues=4
    )
    xpair_d = nc.dram_tensor("xpair", [2 * PSEG, 2 * D], f16i, kind="ExternalInput")
    xself_d = nc.dram_tensor(
        "xself", [128, p.nwin * SELF_Q // 128, D], f16i, kind="ExternalInput"
    )
    wt_d = nc.dram_tensor("wt", [D, D], f16i, kind="ExternalInput")
    iota_d = nc.dram_tensor(
        "iota", [128, p.w_win, p.colsmax], f16i, kind="ExternalInput"
    )
    idx_d = nc.dram_tensor("idx", [128, p.tot_idx_cols], i16, kind="ExternalInput")
    dA_d = nc.dram_tensor("doffA", [128, p.tot_pass], f16i, kind="ExternalInput")
    dB_d = nc.dram_tensor("doffB", [128, p.tot_pass], f16i, kind="ExternalInput")
    sd_d = nc.dram_tensor("sd", [p.w_win, nwin], f32, kind="ExternalInput")
    if p.bias_nonzero:
        sb_d = nc.dram_tensor("sb", [1, nwin * p.w_win], f16i, kind="ExternalInput")
        b_d = nc.dram_tensor("b", [1, D], f16i, kind="ExternalInput")
    out_d = nc.dram_tensor("out", [p.w_win, nwin, D], f32, kind="ExternalOutput")

    segs = [xpair_d[0:PSEG, :], xpair_d[PSEG : 2 * PSEG, :], None]

    with tile.TileContext(nc) as tc:
        with (
            tc.tile_pool(name="const", bufs=1) as constp,
            tc.tile_pool(name="meta", bufs=4) as metap,
            tc.tile_pool(name="msgs", bufs=2) as msgsp,
            tc.tile_pool(name="vh", bufs=2) as vhp,
            tc.tile_pool(name="aggx", bufs=3) as aggxp,
            tc.tile_pool(name="outsb", bufs=2) as outp,
            tc.tile_pool(name="ps1", bufs=4, space="PSUM") as ps1p,
            tc.tile_pool(name="ps2", bufs=2, space="PSUM") as ps2p,
        ):
            wt16 = constp.tile([D, D], f16i, tag="wt16")
            nc.scalar.dma_start(wt16[:], wt_d[:])
            iota_sb = constp.tile([128, p.w_win, p.colsmax], f16i, tag="iota")
            nc.scalar.dma_start(iota_sb[:], iota_d[:])
            sd_sb = constp.tile([p.w_win, nwin], f32, tag="sd")
            nc.scalar.dma_start(sd_sb[:], sd_d[:])
            if p.bias_nonzero:
                b16 = constp.tile([1, D], f16i, tag="b16")
                nc.scalar.dma_start(b16[:], b_d[:])
                sbrow16 = constp.tile([1, nwin * p.w_win], f16i, tag="sbw16")
                nc.scalar.dma_start(sbrow16[:], sb_d[:])

            ic_base = 0
            pass_base = 0
            gq = [0]
            for g in range(p.ngroups):
                w0 = g * group
                gidx_cols = (
                    int(p.B[0, w0 + group] - p.B[0, w0])
                    + int(p.B[1, w0 + group] - p.B[1, w0])
                ) // 16
                gpass = sum(p.gpasses[g])
                idx_sb = metap.tile([128, gidx_cols], i16, tag="idx")
                nc.sync.dma_start(idx_sb[:], idx_d[:, ic_base : ic_base + gidx_cols])
                dA_sb = metap.tile([128, gpass], f16i, tag="dA")
                nc.sync.dma_start(dA_sb[:], dA_d[:, pass_base : pass_base + gpass])
                dB_sb = metap.tile([128, gpass], f16i, tag="dB")
                nc.sync.dma_start(dB_sb[:], dB_d[:, pass_base : pass_base + gpass])

                ms = []
                vhA = []
                vhB = []
                icol = 0
                ppos = 0
                for q in range(nstream):
                    npas = p.gpasses[g][q]
                    if q < 2:
                        total = int(p.B[q, w0 + group] - p.B[q, w0])
                        ncols = total // 128
                        mt = msgsp.tile([128, ncols, 2 * D], f16i, tag=f"m{q}")
                        off = 0
                        while off < total:
                            n = min(GATHER_CAP, total - off)
                            nc.gpsimd.dma_gather(
                                mt[:, off // 128 : (off + n) // 128, :],
                                segs[q],
                                idx_sb[
                                    :, icol + off // 16 : icol + (off + n) // 16
                                ],
                                n,
                                n,
                                2 * D,
                                queue_num=gq[0] % 4,
                                single_packet=False,
                            )
                            gq[0] += 1
                            off += n
                        icol += total // 16
                    else:
                        ncols = group * SELF_Q // 128
                        mt = msgsp.tile([128, ncols, D], f16i, tag="mself")
                        c0 = w0 * SELF_Q // 128
                        nc.sync.dma_start(mt[:], xself_d[:, c0 : c0 + ncols, :])
                    ms.append(mt)

                    def _bcast(ap2d, n=npas):
                        return ap2d.rearrange("p (o c) -> p o c", o=1).broadcast_to(
                            [128, p.w_win, n]
                        )

                    va = vhp.tile([128, p.w_win, npas], f16i, tag=f"va{q}")
                    nc.vector.tensor_tensor(
                        va[:],
                        iota_sb[:, :, :npas],
                        _bcast(dA_sb[:, ppos : ppos + npas]),
                        mybir.AluOpType.is_equal,
                    )
                    vhA.append(va)
                    if q < 2:
                        vb = vhp.tile([128, p.w_win, npas], f16i, tag=f"vb{q}")
                        nc.vector.tensor_tensor(
                            vb[:],
                            iota_sb[:, :, :npas],
                            _bcast(dB_sb[:, ppos : ppos + npas]),
                            mybir.AluOpType.is_equal,
                        )
                        vhB.append(vb)
                    else:
                        vhB.append(None)
                    ppos += npas

                out_sb = outp.tile([p.w_win, group, D], f32, tag="out")
                pass_ctr = [0] * nstream
                for wl in range(group):
                    w = w0 + wl
                    ps1 = ps1p.tile([D, p.w_win], f32, tag="ps1")
                    plist = p.win_passes[w]
                    # expand to (q, col_local, half) matmul list
                    mms = []
                    for pi in plist:
                        _, q, col = p.passes[pi]
                        gbase = (
                            int(p.B[q, w0]) // 128
                            if q < 2
                            else w0 * SELF_Q // 128
                        )
                        col_l = col - gbase
                        pl = pass_ctr[q]
                        pass_ctr[q] += 1
                        mms.append((q, col_l, pl, 0))
                        if q < 2:
                            mms.append((q, col_l, pl, 1))
                    for k, (q, col_l, pl, hf) in enumerate(mms):
                        if q < 2:
                            stat = ms[q][:, col_l, hf * D : (hf + 1) * D]
                            vt = vhA[q] if hf == 0 else vhB[q]
                        else:
                            stat = ms[q][:, col_l, :]
                            vt = vhA[q]
                        nc.tensor.matmul(
                            ps1[:, :],
                            stat,
                            vt[:, :, pl],
                            start=(k == 0),
                            stop=(k == len(mms) - 1),
                        )
                    # pass_ctr counted per window loop; shared straddle cols
                    # advance it once per (q, col) occurrence in plist
                    ag = aggxp.tile([D, p.w_win], f16i, tag="ag")
                    nc.scalar.copy(ag[:], ps1[:])
                    ps2 = ps2p.tile([p.w_win, D], f32, tag="ps2")
                    nc.tensor.matmul(
                        ps2[:, :],
                        ag[:, :],
                        wt16[:, :],
                        start=True,
                        stop=not p.bias_nonzero,
                    )
                    if p.bias_nonzero:
                        nc.tensor.matmul(
                            ps2[:, :],
                            sbrow16[:, w * p.w_win : (w + 1) * p.w_win],
                            b16[:, :],
                            start=False,
                            stop=True,
                        )
                    nc.scalar.activation(
                        out_sb[:, wl, :],
                        ps2[:, :],
                        mybir.ActivationFunctionType.Relu,
                        scale=sd_sb[:, w : w + 1],
                    )
                nc.sync.dma_start(out_d[:, w0 : w0 + group, :], out_sb[:])
                ic_base += gidx_cols
                pass_base += gpass
    nc.compile()
    return nc


def _unshard(p, outs):
    N, D = p.N, p.D
    res = np.empty((N, D), dtype=np.float32)
    for c in range(p.n_cores):
        o = np.asarray(outs[c]).transpose(1, 0, 2).reshape(p.nwin * p.w_win, D)
        memb = p.win_members[c]
        real = memb >= 0
        res[c * p.npc + memb[real]] = o[real]
    return res


def _in_maps(p):
    in_maps = []
    for c in range(p.n_cores):
        m = {
            "xpair": p.xpair_all[c],
            "xself": p.xself_perm[c],
            "wt": p.WT16,
            "iota": p.iota,
            "idx": p.idx_all[c],
            "doffA": p.doffA_all[c],
            "doffB": p.doffB_all[c],
            "sd": p.sd[c],
        }
        if p.bias_nonzero:
            m["sb"] = p.sb[c]
            m["b"] = p.b16
        in_maps.append(m)
    return in_maps


def kernel(x, edge_index, W, b):
    from concourse.bass_utils import run_bass_kernel_spmd

    x = np.asarray(x, dtype=np.float32)
    W = np.asarray(W, dtype=np.float32)
    b = np.asarray(b, dtype=np.float32)
    p = prepare(x, edge_index, W, b)
    nc = build_program(p)
    res = run_bass_kernel_spmd(nc, _in_maps(p), core_ids=list(range(p.n_cores)))
    outs = [r["out"] for r in res.results]
    return _unshard(p, outs)
